# revision 41
# baseline (speedup 1.0000x reference)
"""Trainium2 Bass kernel for nn_Codec (5-level lifting wavelet codec stats).

kernel(**inputs) takes the FULL inputs (x [32,3,512,512] f32 + eight 3-tap
filters) and returns (loss1, loss0, invCR0, invCR1) as np.float32 scalars.

Sharding: pure data parallel - 96 (batch*channel) slices split 12 per core
across 8 NeuronCores; scalar partials are gathered and reduced on the host.

Per-slice device pipeline (v5):
  - Input slices shipped bf16 (host-cast, halves the HBM load).
  - Lifting levels 0-1 entirely on TensorE as bf16 banded matmuls against
    host-composed matrices, with bf16 DMA-xbar transposes between the x and
    y phases. Levels 2-4 (tiny) keep PE x-matmuls + DVE y-convs, all bf16.
  - Subbands land in a [128, 2048] bf16 staging tile.
  - Arithmetic i16 digit extraction on ScalarE (no int bitwise, no casts):
      tr  = rne(relu(v/2 + 2^-10))            == max(floor((v+1)/2), 0)
      d   = v - 2*tr                          (bf16, == fmod deltas for v>-3)
      w   = rne(128 d + 127.5009765625)       == 128 + floor(128 d)
      hd  = rne(w/16 - 0.46875)               == floor(w/16)   (delta)
      ld  = w - 16*hd                         (DVE stt)
    invalid elements (d < -1) give hd < 0 and fall out of every h-plane.
  - RMSE sums via ScalarE Square+accum directly on d (and on x).
  - Joint-count masks: one-hot planes (is_equal) on DVE/GPSIMD + relu RAMP
    planes on ScalarE (basis un-mixed on the host: C = Uh^-1 M Ul^-T).
  - Radix-16x16 (delta) / 16x8 (img) mask planes at FC=1024; joint counts
    via TensorE into PSUM with the G=8 positional-diagonal trick.
"""

import os

import numpy as np
from contextlib import ExitStack

import concourse.bass as bass
import concourse.mybir as mybir
import concourse.tile as tile
from concourse import bacc
from concourse.bass_utils import run_bass_kernel_spmd

F32 = mybir.dt.float32
BF16 = mybir.dt.bfloat16
I16 = mybir.dt.int16
ALU = mybir.AluOpType
ACTF = mybir.ActivationFunctionType

N_CORES = 8
S0 = 512
NSL = 12            # slices per core (96 / 8)
STG = 2048          # staging free dim per slice (512*512/128)
RES = S0 * S0
FC = int(os.environ.get("KB_FC", "1024"))  # mask chunk width (free dim)
N_LEVELS = 5
G = 8               # sub-chunk dup factor for the joint-count matmuls
C0 = 127.5009765625  # 127.5 + 2^-10: rne(v*128 + C0) == 128 + floor(v*128)

# tap vector layout (y-phase DVE convs for levels 2-4 only).
TP_RY, TP_NPY, TP_NCY = 0, 3, 6
NT = 9

# ---------------------------------------------------------------------------
# plane-engine assignment: per side "nD,nP,nA" (DVE one-hot, GPSIMD one-hot,
# ScalarE ramp).  Host reconstruction inverts the mixed basis.
# ---------------------------------------------------------------------------


def _side_assign(n, spec):
    d, p, a = (int(t) for t in spec.split(","))
    assert d + p + a == n, (n, spec)
    return ["D"] * d + ["P"] * p + ["A"] * a


AS_DH = _side_assign(16, os.environ.get("KB_AS_DH", "12,4,0"))
AS_DL = _side_assign(16, os.environ.get("KB_AS_DL", "12,3,1"))
AS_IH = _side_assign(16, os.environ.get("KB_AS_IH", "10,3,3"))
AS_IL = _side_assign(8, os.environ.get("KB_AS_IL", "6,2,0"))


def _basis_matrix(asg):
    """U[j, d] for the device-side plane functions over logical digits d."""
    n = len(asg)
    U = np.zeros((n, n))
    for j, e in enumerate(asg):
        for d in range(n):
            U[j, d] = (1.0 if d == j else 0.0) if e in ("D", "P") else max(d - (j - 1), 0)
    return U


UINV = {
    "dh": np.linalg.inv(_basis_matrix(AS_DH)),
    "dl": np.linalg.inv(_basis_matrix(AS_DL)),
    "ih": np.linalg.inv(_basis_matrix(AS_IH)),
    "il": np.linalg.inv(_basis_matrix(AS_IL)),
}

# optional last-slice override "dh|dl|ih|il" to flatten the Pool-bound
# epilogue (the global split over-assigns GPSIMD per slice; pipelining hides
# that everywhere except the final slice)
_LS = os.environ.get("KB_LAST_AS", "12,2,2|12,2,2|10,3,3|6,2,0")
if _LS:
    _p = _LS.split("|")
    AS_LAST = (_side_assign(16, _p[0]), _side_assign(16, _p[1]),
               _side_assign(16, _p[2]), _side_assign(8, _p[3]))
    UINV_LAST = {
        "dh": np.linalg.inv(_basis_matrix(AS_LAST[0])),
        "dl": np.linalg.inv(_basis_matrix(AS_LAST[1])),
        "ih": np.linalg.inv(_basis_matrix(AS_LAST[2])),
        "il": np.linalg.inv(_basis_matrix(AS_LAST[3])),
    }
else:
    AS_LAST = None
    UINV_LAST = UINV

YPSUM_BUFS = int(os.environ.get("KB_YPSUM_BUFS", "3"))
MASK_BUFS = int(os.environ.get("KB_MASK_BUFS", "2"))
LIFT_BUFS = int(os.environ.get("KB_LIFT_BUFS", "3"))
LIFT_COPY_DVE = os.environ.get("KB_LIFT_COPY_DVE", "")  # e.g. "x0,y0,x1,y1"
SCRATCH_BUFS = int(os.environ.get("KB_SCRATCH_BUFS", "1"))
PREP_DVE = os.environ.get("KB_PREP_DVE", "") # subset of "wi,hi,wd,hd"

# ---------------------------------------------------------------------------
# host-side matrix composition + block plans
# ---------------------------------------------------------------------------


def _make_mats(S, p, u, c, r, update):
    """A (odd out) and B (even out) lifting matrices [S/2, S], f64->f32.
    update=False omits the update step (y-lifting at levels >= 2)."""
    half = S // 2
    E = np.zeros((half, S))
    O = np.zeros((half, S))
    E[np.arange(half), 2 * np.arange(half)] = 1.0
    O[np.arange(half), 2 * np.arange(half) + 1] = 1.0

    def T(k):
        M = np.zeros((half, half))
        i = np.arange(half)
        M[i, i] = k[1]
        M[i[1:], i[1:] - 1] = k[0]
        M[i[:-1], i[:-1] + 1] = k[2]
        return M

    o1 = O - T(p.astype(np.float64)) @ E
    e1 = E + T(u.astype(np.float64)) @ o1 if update else E
    A = o1 - T(c.astype(np.float64)) @ e1
    B = e1 + T(r.astype(np.float64)) @ A
    return A.astype(np.float32), B.astype(np.float32)


def _plan(S):
    """Static nonzero-block structure for a [S/2, S] composed lifting matrix
    (band halfwidth <= 10 in the S domain): per out-tile r, the list of
    128-wide K-block cols that are structurally nonzero."""
    half = S // 2
    t_out = max(1, (half + 127) // 128)
    t_in = max(1, (S + 127) // 128)
    rows = []
    for r in range(t_out):
        m0 = 128 * r
        m1 = min(m0 + 128, half)
        j0 = max(0, 2 * m0 - 10)
        j1 = min(S - 1, 2 * (m1 - 1) + 10)
        rows.append([c for c in range(t_in) if 128 * c <= j1 and 128 * c + 127 >= j0])
    return rows


X_PLANS = [_plan(S0 >> lvl) for lvl in range(N_LEVELS)]
Y_PLANS = [_plan(S0 >> lvl) for lvl in range(2)]  # y on PE for lvl 0-1 only

# wyb (bf16) block order: x lvl0 A/B, y lvl0 C/R, x lvl1 A/B, y lvl1 C/R,
# x lvl2 A/B, x lvl3 A/B, x lvl4 A/B
NWB = (
    2 * sum(len(cs) for cs in X_PLANS[0])
    + 2 * sum(len(cs) for cs in Y_PLANS[0])
    + 2 * sum(len(cs) for cs in X_PLANS[1])
    + 2 * sum(len(cs) for cs in Y_PLANS[1])
    + 2 * sum(len(cs) for cs in X_PLANS[2])
    + 2 * sum(len(cs) for cs in X_PLANS[3])
    + 2 * sum(len(cs) for cs in X_PLANS[4])
)


def _pack_blocks(M_, plan, buf, i):
    half, S = M_.shape[0], M_.shape[1]
    for r, cs in enumerate(plan):
        m0, m1 = 128 * r, min(128 * r + 128, half)
        for c in cs:
            k0, k1 = 128 * c, min(128 * c + 128, S)
            buf[i, : k1 - k0, : m1 - m0] = M_[m0:m1, k0:k1].T
            i += 1
    return i


def _check_cover(M_, plan):
    half, S = M_.shape
    mass = np.abs(M_).sum()
    cov = 0.0
    for r, cs in enumerate(plan):
        m0, m1 = 128 * r, min(128 * r + 128, half)
        for c in cs:
            k0, k1 = 128 * c, min(128 * c + 128, S)
            cov += np.abs(M_[m0:m1, k0:k1]).sum()
    assert abs(cov - mass) < 1e-6 * max(mass, 1), (half, S, cov, mass)


def _build_w_host(px, ux, cx, rx, py, uy, cy, ry):
    xmats = [_make_mats(S0 >> l, px, ux, cx, rx, update=True) for l in range(N_LEVELS)]
    ymats = [_make_mats(S0 >> l, py, uy, cy, ry, update=(l < 2)) for l in range(2)]
    for lvl in range(N_LEVELS):
        for M_ in xmats[lvl]:
            _check_cover(M_, X_PLANS[lvl])
    for lvl in range(2):
        for M_ in ymats[lvl]:
            _check_cover(M_, Y_PLANS[lvl])

    wyb = np.zeros((NWB, 128, 128), np.float32)
    i = 0
    for M_ in xmats[0]:
        i = _pack_blocks(M_, X_PLANS[0], wyb, i)
    for M_ in ymats[0]:
        i = _pack_blocks(M_, Y_PLANS[0], wyb, i)
    for M_ in xmats[1]:
        i = _pack_blocks(M_, X_PLANS[1], wyb, i)
    for M_ in ymats[1]:
        i = _pack_blocks(M_, Y_PLANS[1], wyb, i)
    for lvl in (2, 3, 4):
        for M_ in xmats[lvl]:
            i = _pack_blocks(M_, X_PLANS[lvl], wyb, i)
    assert i == NWB, (i, NWB)
    return wyb


# staging slots for the deep subbands (levels 2-4). (p0, p1, c0, c1)
DEEP_SLOTS = {
    "l2xo2": (0, 64, 1920, 2048),
    "l2yo2": (64, 128, 1920, 1984),
    "l3xo2": (64, 96, 1984, 2048),
    "l3yo2": (96, 128, 1984, 2016),
    "l4xo2": (96, 112, 2016, 2048),
    "l4yo2": (112, 128, 2016, 2032),
    "ye4": (112, 128, 2032, 2048),
}

# ---------------------------------------------------------------------------
# device kernel
# ---------------------------------------------------------------------------


def _conv_step(nc, out_ap, base_ap, src_ap, tap_col, tp_sb, P, F):
    """out = base + 3-tap conv of src along the free dim, zero padding."""
    k0 = tp_sb[0:P, tap_col : tap_col + 1]
    k1 = tp_sb[0:P, tap_col + 1 : tap_col + 2]
    k2 = tp_sb[0:P, tap_col + 2 : tap_col + 3]
    nc.vector.scalar_tensor_tensor(out_ap, src_ap, k1, base_ap, ALU.mult, ALU.add)
    nc.vector.scalar_tensor_tensor(
        out_ap[:, 1:F], src_ap[:, 0 : F - 1], k0, out_ap[:, 1:F], ALU.mult, ALU.add
    )
    nc.vector.scalar_tensor_tensor(
        out_ap[:, 0 : F - 1], src_ap[:, 1:F], k2, out_ap[:, 0 : F - 1], ALU.mult, ALU.add
    )


def _emit_planes(nc, mask_ap, src_flat, src_grp, asg, base, rbias, n_mm,
                 idxg=None):
    """Write the one-hot / ramp planes for one digit side of one chunk.

    mask_ap: [128, n_mm, n*G] destination
    src_flat: [128, FC] digit tensor (i16), src_grp: grouped view
    asg: list of 'D'/'P'/'A' per plane
    base: digit value of plane 0 (16 for the img h side, else 0)
    idxg: [128, NIDX, G] bf16 const with value k at [:, k, :] -- when given,
    the contiguous 'P' block is emitted as ONE gpsimd tensor_tensor against
    the broadcast pattern (saves per-op Q7 launches).
    """
    pj = [j for j, e in enumerate(asg) if e == "P"]
    merged_p = idxg is not None and len(pj) > 1 and pj == list(
        range(pj[0], pj[0] + len(pj)))
    for j, e in enumerate(asg):
        dst = mask_ap[:, :, j * G : (j + 1) * G]
        if e == "A":
            # ramp: relu(digit - (base + j - 1))
            b = -(base + j - 1)
            nc.scalar.activation(dst, src_flat, ACTF.Relu, bias=rbias(b))
        elif e == "P":
            if merged_p:
                if j != pj[0]:
                    continue
                npl = len(pj)
                q0 = base + pj[0]
                dstm = mask_ap[:, :, pj[0] * G : (pj[0] + npl) * G].rearrange(
                    "p n (q g) -> p n q g", g=G)
                in0 = src_grp.rearrange("p n (o g) -> p n o g", o=1).broadcast_to(
                    [128, n_mm, npl, G])
                in1 = idxg[:, q0 : q0 + npl, :].rearrange(
                    "p (o q) g -> p o q g", o=1).broadcast_to([128, n_mm, npl, G])
                nc.gpsimd.tensor_tensor(dstm, in0, in1, ALU.is_equal)
            else:
                nc.gpsimd.tensor_scalar(dst, src_grp, float(base + j), None,
                                        ALU.is_equal)
        else:
            nc.vector.tensor_scalar(dst, src_grp, float(base + j), None, ALU.is_equal)


def _hist_pipeline(nc, mpool, hb_src, lb_src, asg_h, asg_l, h_base, psum_ap,
                   rbias, idxg=None):
    """One-hot/ramp mask planes + joint-count matmuls over [128, FC] i16
    digit tensors for one chunk."""
    n_h, n_l = len(asg_h), len(asg_l)
    n_mm = FC // G
    hb_grp = hb_src.rearrange("p (n g) -> p n g", g=G)
    lb_grp = lb_src.rearrange("p (n g) -> p n g", g=G)
    # fixed 16*G width regardless of n_l so the pool tag cycles consistently
    mh_t = mpool.tile([128, n_mm, 16 * G], BF16, tag="mh")
    ml_t = mpool.tile([128, n_mm, 16 * G], BF16, tag="ml")
    _emit_planes(nc, mh_t, hb_src, hb_grp, asg_h, h_base, rbias, n_mm, idxg)
    _emit_planes(nc, ml_t, lb_src, lb_grp, asg_l, 0, rbias, n_mm, idxg)
    return mh_t, ml_t, n_mm


def build_nc(nsl=NSL):
    nc = bacc.Bacc("TRN2", target_bir_lowering=False, debug=False)
    xs = nc.dram_tensor("xs", [nsl, S0, S0], BF16, kind="ExternalInput")
    tp = nc.dram_tensor("tp", [NT], F32, kind="ExternalInput")
    wyb = nc.dram_tensor("wyb", [NWB, 128, 128], BF16, kind="ExternalInput")
    pd = nc.dram_tensor("pd", [nsl, 128, 128], F32, kind="ExternalOutput")
    pi = nc.dram_tensor("pi", [nsl, 128, 64], F32, kind="ExternalOutput")
    accd = nc.dram_tensor("accd", [128, nsl * 8], F32, kind="ExternalOutput")
    dbg_stg = os.environ.get("KB_DBG_STG") == "1"
    stgd = (nc.dram_tensor("stgd", [nsl, 128, STG], BF16, kind="ExternalOutput")
            if dbg_stg else None)
    dbg_d = os.environ.get("KB_DBG_D") == "1"
    dd = (nc.dram_tensor("dd", [nsl, 128, STG], BF16, kind="ExternalOutput")
          if dbg_d else None)

    lift_copy_dve = set(t for t in LIFT_COPY_DVE.split(",") if t)

    def lift_copy(dst, src, site):
        if site in lift_copy_dve:
            nc.vector.tensor_copy(dst, src)
        else:
            nc.scalar.copy(dst, src)

    with tile.TileContext(nc) as tc:
        with ExitStack() as ctx:
            const = ctx.enter_context(tc.tile_pool(name="const", bufs=1))
            xpool = ctx.enter_context(tc.tile_pool(name="xpool", bufs=int(os.environ.get("KB_XPOOL_BUFS", "2"))))
            stgp = ctx.enter_context(tc.tile_pool(name="stgp", bufs=int(os.environ.get("KB_STG_BUFS", "2"))))
            lift = ctx.enter_context(tc.tile_pool(name="lift", bufs=LIFT_BUFS))
            work = ctx.enter_context(tc.tile_pool(name="work", bufs=2))
            liftb = ctx.enter_context(tc.tile_pool(name="liftb", bufs=1))
            scratch = ctx.enter_context(tc.tile_pool(name="scratch", bufs=SCRATCH_BUFS))
            maskp = ctx.enter_context(tc.tile_pool(name="masks", bufs=MASK_BUFS))
            psum = ctx.enter_context(tc.tile_pool(name="psum", bufs=int(os.environ.get("KB_PSUM_BUFS", "2")), space="PSUM"))
            ypsum = ctx.enter_context(tc.tile_pool(name="ypsum", bufs=YPSUM_BUFS, space="PSUM"))

            acc = const.tile([128, nsl * 8], F32)
            nc.vector.memset(acc[:], 0.0)
            tp_sb = const.tile([128, NT], F32)
            nc.sync.dma_start(
                tp_sb[:], tp.ap().rearrange("(o n) -> o n", o=1).broadcast_to([128, NT])
            )
            wyb_sb = const.tile([128, NWB * 128], BF16)
            nb_split = (2 * sum(len(cs) for cs in X_PLANS[0])
                        if os.environ.get("KB_WYB_SPLIT", "0") == "1" else NWB)
            nc.sync.dma_start(
                wyb_sb[:, 0 : nb_split * 128].rearrange("k (n m) -> k n m", n=nb_split),
                wyb.ap()[0:nb_split].rearrange("n k m -> k n m"),
            )
            if nb_split < NWB:
                nc.sync.dma_start(
                    wyb_sb[:, nb_split * 128 :].rearrange("k (n m) -> k n m", n=NWB - nb_split),
                    wyb.ap()[nb_split:NWB].rearrange("n k m -> k n m"),
                )

            # activation bias constants:
            #   cpre[0] = C0, cpre[1] = 2^-10 (relu-trunc), cpre[2] = -0.46875
            #   (floor /16), cpre[3] = -0.4375 (floor /8)
            cpre = const.tile([128, 4], F32, tag="cpre")
            for k, v in enumerate((C0, 0.0009765625, -0.46875, -0.4375)):
                nc.vector.memset(cpre[:, k : k + 1], v)
            # ramp biases: integer values -31..+1 at slot v+31
            rb = const.tile([128, 33], F32, tag="rb")
            for k in range(33):
                nc.vector.memset(rb[:, k : k + 1], float(k - 31))

            def rbias(v):
                k = int(v) + 31
                assert 0 <= k < 33, v
                return rb[:, k : k + 1]

            if os.environ.get("KB_POOL_MERGE", "0") == "1":
                idxg = const.tile([128, 32, G], I16, tag="idxg")
                for k in range(32):
                    nc.vector.memset(idxg[:, k, :], float(k))
            else:
                idxg = None


            nb_x0 = 2 * sum(len(cs) for cs in X_PLANS[0])
            nb_y0 = nb_x0 + 2 * sum(len(cs) for cs in Y_PLANS[0])
            nb_x1 = nb_y0 + 2 * sum(len(cs) for cs in X_PLANS[1])
            nb_y1 = nb_x1 + 2 * sum(len(cs) for cs in Y_PLANS[1])

            def lift_a0(st):
                s = st["s"]
                # ---- load slice (transposed, bf16): x_sb[p, t, h]
                x_sb = xpool.tile([128, 4, S0], BF16, tag="x_sb")
                nc.sync.dma_start(
                    x_sb[:], xs.ap()[s].rearrange("(t p) w -> p t w", p=128)
                )
                st["x_sb"] = x_sb
                st["x_flat"] = x_sb[:].rearrange("p t w -> p (t w)")

                stg = stgp.tile([128, STG], BF16, tag="stg")
                st["stg"] = stg
                # ================= level 0 (PE, bf16) =====================
                xe2b = liftb.tile([128, 2, S0], BF16, tag="xe2b0")
                st["xe2b"] = xe2b
                wf = 0
                for kind in range(2):  # 0: A (xo), 1: B (xe2)
                    for r, cs in enumerate(X_PLANS[0]):
                        ps = ypsum.tile([128, 512], F32, tag="yps")
                        for i, c in enumerate(cs):
                            nc.tensor.matmul(
                                ps[:, :],
                                wyb_sb[0:128, 128 * wf : 128 * wf + 128],
                                x_sb[:, c, :],
                                start=(i == 0),
                                stop=(i == len(cs) - 1),
                                skip_group_check=True,
                            )
                            wf += 1
                        if kind == 0:
                            lift_copy(stg[:, 512 * r : 512 * (r + 1)], ps[:, :], "x0")
                        else:
                            lift_copy(xe2b[:, r, :], ps[:, :], "x0")
                assert wf == nb_x0

            def lift_a1(st):
                stg, xe2b = st["stg"], st["xe2b"]
                # transpose xe2 [256, 512] -> xe2T (bf16 xbar)
                xe2T = lift.tile([128, 2, 4, 128], BF16, tag="xe2T0")
                for r in range(2):
                    nc.sync.dma_start_transpose(xe2T[:, r, :, :], xe2b[:, r, :])

                # y-phase: yo = C0 @ xe2T, ye2 = R0 @ xe2T, per (r2, r)
                ye2b = liftb.tile([128, 2, 2, 128], BF16, tag="ye2b0")
                wb = nb_x0
                for kind in range(2):  # 0: C (yo), 1: R (ye2)
                    for r2, cs in enumerate(Y_PLANS[0]):
                        blk0 = wb
                        for r in range(2):
                            ps2f = ypsum.tile([128, 512], F32, tag="yps")
                            ps2 = ps2f[:, 0:128]
                            for i, c2 in enumerate(cs):
                                nc.tensor.matmul(
                                    ps2,
                                    wyb_sb[0:128, 128 * (blk0 + i) : 128 * (blk0 + i) + 128],
                                    xe2T[:, r, c2, :],
                                    start=(i == 0),
                                    stop=(i == len(cs) - 1),
                                    skip_group_check=True,
                                )
                            if kind == 0:
                                col = 1024 + 128 * (2 * r2 + r)
                                lift_copy(stg[:, col : col + 128], ps2, "y0")
                            else:
                                lift_copy(ye2b[:, r2, r, :], ps2, "y0")
                        wb = blk0 + len(cs)
                assert wb == nb_y0

                cur1 = lift.tile([128, 2, 256], BF16, tag="cur1")
                st["cur1"] = cur1
                for r2 in range(2):
                    for r in range(2):
                        nc.sync.dma_start_transpose(
                            cur1[:, r, 128 * r2 : 128 * r2 + 128], ye2b[:, r2, r, :]
                        )

            def lift_a2(st):
                stg, cur1 = st["stg"], st["cur1"]
                # ================= level 1 (PE bf16) ======================
                xe2b1 = liftb.tile([128, 256], BF16, tag="xe2b1")
                for kind in range(2):
                    base = nb_y0 + (0 if kind == 0 else len(X_PLANS[1][0]))
                    psf = ypsum.tile([128, 512], F32, tag="yps")
                    ps = psf[:, 0:256]
                    cs = X_PLANS[1][0]
                    for i, c in enumerate(cs):
                        nc.tensor.matmul(
                            ps,
                            wyb_sb[0:128, 128 * (base + i) : 128 * (base + i) + 128],
                            cur1[:, c, :],
                            start=(i == 0),
                            stop=(i == len(cs) - 1),
                            skip_group_check=True,
                        )
                    if kind == 0:
                        lift_copy(stg[:, 1536:1792], ps, "x1")
                    else:
                        lift_copy(xe2b1[:], ps, "x1")

                xe2T1 = lift.tile([128, 2, 128], BF16, tag="xe2T1")
                nc.sync.dma_start_transpose(xe2T1[:, :, :], xe2b1[:])

                ye2b1 = liftb.tile([128, 128], BF16, tag="ye2b1")
                for kind in range(2):
                    base = nb_x1 + (0 if kind == 0 else len(Y_PLANS[1][0]))
                    ps2f = ypsum.tile([128, 512], F32, tag="yps")
                    ps2 = ps2f[:, 0:128]
                    cs = Y_PLANS[1][0]
                    for i, c2 in enumerate(cs):
                        nc.tensor.matmul(
                            ps2,
                            wyb_sb[0:128, 128 * (base + i) : 128 * (base + i) + 128],
                            xe2T1[:, c2, :],
                            start=(i == 0),
                            stop=(i == len(cs) - 1),
                            skip_group_check=True,
                        )
                    if kind == 0:
                        lift_copy(stg[:, 1792:1920], ps2, "y1")
                    else:
                        lift_copy(ye2b1[:], ps2, "y1")

                cur2 = lift.tile([128, 128], BF16, tag="cur2")
                st["cur2"] = cur2
                nc.sync.dma_start_transpose(cur2[:, :], ye2b1[:])

            def lift_a3(st):
                stg, cur2 = st["stg"], st["cur2"]
                # ================= levels 2-4 (PE x bf16, DVE y) ==========
                cur_tiles = [(cur2[:, :], 128)]
                wb2 = nb_y1
                for lvl in range(2, N_LEVELS):
                    S = S0 >> lvl
                    half = S // 2
                    xe2_tiles = []
                    for kind in range(2):
                        cs = X_PLANS[lvl][0]
                        M = half
                        psf = ypsum.tile([128, 512], F32, tag="yps")
                        ps = psf[0:M, 0:S]
                        for i, c in enumerate(cs):
                            ap, K = cur_tiles[c]
                            nc.tensor.matmul(
                                ps,
                                wyb_sb[0:K, 128 * wb2 : 128 * wb2 + M],
                                ap,
                                start=(i == 0),
                                stop=(i == len(cs) - 1),
                                skip_group_check=True,
                            )
                            wb2 += 1
                        if kind == 0:
                            if lvl == 2:
                                p0, p1, q0, q1 = DEEP_SLOTS["l2xo2"]
                                nc.scalar.copy(stg[p0:p1, q0:q1], ps)
                            else:
                                key = "l3xo2" if lvl == 3 else "l4xo2"
                                p0, p1, q0, q1 = DEEP_SLOTS[key]
                                xo2s = work.tile([M, S], BF16, tag=f"xo2s_{lvl}")
                                nc.scalar.copy(xo2s[:], ps)
                                nc.sync.dma_start(stg[p0:p1, q0:q1], xo2s[:])
                        else:
                            xe2 = work.tile([M, S], BF16, tag=f"xe2_{lvl}")
                            nc.scalar.copy(xe2[:], ps)
                            xe2_tiles.append((xe2, M))

                    new_cur = []
                    for xe2, P in xe2_tiles:
                        ye_v = xe2[0:P, 0:S:2]
                        yo_v = xe2[0:P, 1:S:2]
                        yo1 = work.tile([P, half], BF16, tag=f"yo1_{lvl}")
                        _conv_step(nc, yo1[:], yo_v, ye_v, TP_NPY, tp_sb, P, half)
                        yo2_t = work.tile([P, half], BF16, tag=f"yo2_{lvl}")
                        _conv_step(nc, yo2_t[:], yo1[:], ye_v, TP_NCY, tp_sb, P, half)
                        key = {2: "l2yo2", 3: "l3yo2", 4: "l4yo2"}[lvl]
                        p0, p1, q0, q1 = DEEP_SLOTS[key]
                        nc.sync.dma_start(stg[p0:p1, q0:q1], yo2_t[:])
                        ye2 = work.tile([P, half], BF16, tag=f"ye2_{lvl}")
                        _conv_step(nc, ye2[:], ye_v, yo2_t[:], TP_RY, tp_sb, P, half)
                        if lvl < N_LEVELS - 1:
                            new_cur.append((ye2[:], P))
                        else:
                            p0, p1, q0, q1 = DEEP_SLOTS["ye4"]
                            nc.sync.dma_start(stg[p0:p1, q0:q1], ye2[:])
                    cur_tiles = new_cur


            NCH = STG // FC
            assert NCH == 2
            prep_dve = set(t for t in PREP_DVE.split(","))

            def img_prep(st, ch):
                s, x_flat = st["s"], st["x_flat"]
                sc = scratch
                lo, hi = ch * FC, (ch + 1) * FC
                xv = x_flat[:, lo:hi]
                w16 = sc.tile([128, FC], I16, tag=f"wi{ch}")
                if "wi" in prep_dve:
                    nc.vector.tensor_scalar(w16[:], xv, 128.0, C0, ALU.mult, ALU.add)
                else:
                    nc.scalar.activation(w16[:], xv, ACTF.Identity, scale=128.0,
                                         bias=cpre[:, 0:1])
                hd16 = sc.tile([128, FC], I16, tag=f"hi{ch}")
                if "hi" in prep_dve:
                    nc.vector.tensor_scalar(hd16[:], w16[:], 0.125, -0.4375,
                                            ALU.mult, ALU.add)
                else:
                    nc.scalar.activation(hd16[:], w16[:], ACTF.Identity, scale=0.125,
                                         bias=cpre[:, 3:4])
                ld16 = sc.tile([128, FC], I16, tag=f"li{ch}")
                nc.vector.scalar_tensor_tensor(
                    ld16[:], hd16[:], -8.0, w16[:], ALU.mult, ALU.add
                )
                junk = sc.tile([128, FC], BF16, tag="J")
                nc.scalar.activation(
                    junk[:], xv, ACTF.Square,
                    accum_out=acc[:, s * 8 + 4 + ch : s * 8 + 5 + ch],
                )
                st[f"ihl{ch}"] = (hd16, ld16)

            def img_mm(st, ch):
                if ch == 0:
                    ps_if = psum.tile([128, 128], F32, tag="ps")
                    st["ps_i"] = ps_if[:, 0:64]
                hd16, ld16 = st[f"ihl{ch}"]
                a_ih, a_il = ((AS_LAST[2], AS_LAST[3])
                              if AS_LAST and st["s"] == nsl - 1
                              else (AS_IH, AS_IL))
                mh, ml, n_mm = _hist_pipeline(
                    nc, maskp, hd16[:], ld16[:], a_ih, a_il, 16, None, rbias,
                    idxg,
                )
                ps_i = st["ps_i"]
                for g_ in range(n_mm):
                    nc.tensor.matmul(
                        ps_i,
                        mh[:, g_, 0 : 16 * G],
                        ml[:, g_, 0 : 8 * G],
                        start=(ch == 0 and g_ == 0),
                        stop=(ch == NCH - 1 and g_ == n_mm - 1),
                        skip_group_check=True,
                    )
                if ch == NCH - 1:
                    pi_sb = work.tile([128, 64], F32, tag="pi_sb")
                    nc.scalar.copy(pi_sb[:], st["ps_i"])
                    nc.sync.dma_start(pi.ap()[st["s"]], pi_sb[:])

            def delta_prep(st, ch):
                s, stg = st["s"], st["stg"]
                sc = scratch
                lo, hi = ch * FC, (ch + 1) * FC
                sv = stg[:, lo:hi]
                if ch == 0 and dbg_stg:
                    nc.sync.dma_start(stgd.ap()[s], stg[:])
                tr16 = sc.tile([128, FC], I16, tag=f"tr{ch}")
                nc.scalar.activation(tr16[:], sv, ACTF.Relu, scale=0.5,
                                     bias=cpre[:, 1:2])
                d = sc.tile([128, FC], BF16, tag=f"d{ch}")
                nc.vector.scalar_tensor_tensor(
                    d[:], tr16[:], -2.0, sv, ALU.mult, ALU.add
                )
                w16 = sc.tile([128, FC], I16, tag=f"wd{ch}")
                if "wd" in prep_dve:
                    nc.vector.tensor_scalar(w16[:], d[:], 128.0, C0, ALU.mult, ALU.add)
                else:
                    nc.scalar.activation(w16[:], d[:], ACTF.Identity, scale=128.0,
                                         bias=cpre[:, 0:1])
                hd16 = sc.tile([128, FC], I16, tag=f"hd{ch}")
                if "hd" in prep_dve:
                    nc.vector.tensor_scalar(hd16[:], w16[:], 0.0625, -0.46875,
                                            ALU.mult, ALU.add)
                else:
                    nc.scalar.activation(hd16[:], w16[:], ACTF.Identity, scale=0.0625,
                                         bias=cpre[:, 2:3])
                ld16 = sc.tile([128, FC], I16, tag=f"ld{ch}")
                nc.vector.scalar_tensor_tensor(
                    ld16[:], hd16[:], -16.0, w16[:], ALU.mult, ALU.add
                )
                junk = sc.tile([128, FC], BF16, tag="J")
                nc.scalar.activation(
                    junk[:], d[:], ACTF.Square,
                    accum_out=acc[:, s * 8 + ch : s * 8 + 1 + ch],
                )
                if dbg_d:
                    nc.sync.dma_start(dd.ap()[s][:, lo:hi], d[:])
                st[f"dhl{ch}"] = (hd16, ld16)

            def delta_mm(st, ch):
                if ch == 0:
                    ps_df = psum.tile([128, 128], F32, tag="ps")
                    st["ps_d"] = ps_df[:, :]
                hd16, ld16 = st[f"dhl{ch}"]
                a_dh, a_dl = ((AS_LAST[0], AS_LAST[1])
                              if AS_LAST and st["s"] == nsl - 1
                              else (AS_DH, AS_DL))
                mh, ml, n_mm = _hist_pipeline(
                    nc, maskp, hd16[:], ld16[:], a_dh, a_dl, 0, None, rbias,
                    idxg,
                )
                ps_d = st["ps_d"]
                for g_ in range(n_mm):
                    nc.tensor.matmul(
                        ps_d,
                        mh[:, g_, 0 : 16 * G],
                        ml[:, g_, 0 : 16 * G],
                        start=(ch == 0 and g_ == 0),
                        stop=(ch == NCH - 1 and g_ == n_mm - 1),
                        skip_group_check=True,
                    )
                if ch == NCH - 1:
                    pd_sb = work.tile([128, 128], F32, tag="pd_sb")
                    nc.scalar.copy(pd_sb[:], st["ps_d"])
                    nc.sync.dma_start(pd.ap()[st["s"]], pd_sb[:])

            A_ST = [lift_a0, lift_a1, lift_a2, lift_a3]
            P_ST = [lambda st: img_prep(st, 0), lambda st: img_prep(st, 1),
                    lambda st: delta_prep(st, 0), lambda st: delta_prep(st, 1)]
            M_ST = [lambda st: img_mm(st, 0), lambda st: img_mm(st, 1),
                    lambda st: delta_mm(st, 0), lambda st: delta_mm(st, 1)]

            ORDERS = {
                # old-pipe replica: full lift, then preps, then masks+mms
                "old": "A0 A1 A2 A3 P0 P1 P2 P3 M0 M1 M2 M3",
                "pfirst": "P0 P1 P2 P3 A0 A1 A2 A3 M0 M1 M2 M3",
                "pmix": "P2 P3 A0 P0 P1 A1 M0 A2 M1 A3 M2 M3",
                "mmix": "A0 P0 P1 P2 P3 M0 A1 M1 A2 M2 A3 M3",
                "fine": "A0 P0 M0 A1 P1 M1 A2 P2 M2 A3 P3 M3",
                "oldx": "A0 A1 A2 A3 P0 P1 P2 P3 M0 M2 M1 M3",
                "oldy": "A0 A1 A2 A3 P2 P3 P0 P1 M2 M0 M3 M1",
                "oldz": "A0 A1 A2 A3 P0 P2 P1 P3 M0 M2 M1 M3",
                "oldw": "A0 P0 P1 P2 P3 A1 A2 A3 M0 M1 M2 M3",
                "oldv": "A0 P2 P3 P0 P1 A1 A2 A3 M0 M1 M2 M3",
                "oldu": "A0 P0 P1 P2 P3 M0 A1 A2 A3 M1 M2 M3",
            }
            ordname = os.environ.get("KB_ORD", "old")
            if ordname == "half":
                # half-slice offset: img hist of slice s in iteration s,
                # delta hist of slice s in iteration s+1
                prev = None
                for s in range(nsl):
                    st = {"s": s}
                    for k in range(4):
                        A_ST[k](st)
                    P_ST[0](st)
                    P_ST[1](st)
                    if prev is not None:
                        P_ST[2](prev)
                        P_ST[3](prev)
                    M_ST[0](st)
                    M_ST[1](st)
                    if prev is not None:
                        M_ST[2](prev)
                        M_ST[3](prev)
                    prev = st
                for k in (2, 3):
                    P_ST[k](prev)
                for k in (2, 3):
                    M_ST[k](prev)
            else:
                order = ORDERS[ordname].split()
                early0 = os.environ.get("KB_EARLY0", "0") == "1"
                prev = None
                for s in range(nsl):
                    st = {"s": s}
                    for tok in order:
                        k = int(tok[1])
                        if tok[0] == "A":
                            A_ST[k](st)
                        elif prev is not None:
                            if prev.get("img_done") and tok in (
                                    "P0", "P1", "M0", "M1"):
                                continue
                            (P_ST if tok[0] == "P" else M_ST)[k](prev)
                    if early0 and s == 0:
                        # slice 0's img half needs only x_sb(0): start it now
                        # to fill the otherwise-idle first iteration
                        P_ST[0](st)
                        P_ST[1](st)
                        M_ST[0](st)
                        M_ST[1](st)
                        st["img_done"] = True
                    prev = st
                for tok in order:
                    k = int(tok[1])
                    if tok[0] == "P":
                        P_ST[k](prev)
                    elif tok[0] == "M":
                        M_ST[k](prev)

            nc.sync.dma_start(accd.ap()[:, :], acc[:])

    nc.compile()
    return nc


_NC_CACHE = {}


def _get_nc():
    if "nc" not in _NC_CACHE:
        _NC_CACHE["nc"] = build_nc()
    return _NC_CACHE["nc"]


LAST_INFO = {}


def kernel(x, px, ux, cx, rx, py, uy, cy, ry, _trace=False):
    import ml_dtypes

    x = np.asarray(x, dtype=np.float32)
    px, ux, cx, rx, py, uy, cy, ry = (
        np.asarray(k, dtype=np.float32) for k in (px, ux, cx, rx, py, uy, cy, ry)
    )

    nc = _get_nc()

    tp_host = np.zeros(NT, np.float32)
    tp_host[TP_RY : TP_RY + 3] = ry
    tp_host[TP_NPY : TP_NPY + 3] = -py
    tp_host[TP_NCY : TP_NCY + 3] = -cy
    wyb_host = _build_w_host(px, ux, cx, rx, py, uy, cy, ry).astype(ml_dtypes.bfloat16)

    # W-major (transposed) slices, cast bf16 on the host
    shards = np.ascontiguousarray(
        x.reshape(N_CORES, NSL, S0, S0).transpose(0, 1, 3, 2)
    ).astype(ml_dtypes.bfloat16)
    in_maps = [
        {"xs": np.ascontiguousarray(shards[i]), "tp": tp_host, "wyb": wyb_host}
        for i in range(N_CORES)
    ]
    if not _trace:
        os.environ.setdefault("BASS_NEVER_TRACE", "1")
    res = run_bass_kernel_spmd(nc, in_maps, core_ids=list(range(N_CORES)), trace=_trace)
    LAST_INFO["exec_time_ns"] = res.exec_time_ns
    LAST_INFO["results"] = res

    counts_img = np.zeros((96, 256))
    counts_delta = np.zeros((96, 256))
    ss_delta_t = 0.0
    ss_img_t = 0.0
    for core in range(N_CORES):
        out = res.results[core]
        pd_ = out["pd"].astype(np.float64)
        pi_ = out["pi"].astype(np.float64)
        acc_ = out["accd"].astype(np.float64).sum(axis=0)
        for s in range(NSL):
            gs = core * NSL + s
            ss_delta_t += acc_[s * 8] + acc_[s * 8 + 1]
            ss_img_t += acc_[s * 8 + 4] + acc_[s * 8 + 5]
            md = np.einsum("afbf->ab", pd_[s].reshape(16, 8, 16, 8))
            mi = np.einsum("afbf->ab", pi_[s].reshape(16, 8, 8, 8))
            uv = UINV_LAST if s == NSL - 1 else UINV
            cd = np.maximum(np.rint(uv["dh"] @ md @ uv["dl"].T), 0.0)
            ci = np.maximum(np.rint(uv["ih"] @ mi @ uv["il"].T), 0.0)
            counts_delta[gs] = cd.reshape(256)
            counts_img[gs, 128:256] = ci.reshape(128)

    loss1 = np.float32(255.0 * np.sqrt(ss_delta_t / (96 * RES)))
    loss0 = np.float32(255.0 * np.sqrt(ss_img_t / (96 * RES)))

    def ent(counts):
        p = counts / RES
        pz = np.where(p > 0, p, 1.0)
        return float(np.sum(-p * np.log2(pz)))

    invCR0 = np.float32(ent(counts_img) / (8.0 * 96))
    invCR1 = np.float32(ent(counts_delta) / (8.0 * 96))
    LAST_INFO.update(
        counts_img=counts_img, counts_delta=counts_delta, ss_img=ss_img_t,
        ss_delta=ss_delta_t,
    )
    return loss1, loss0, invCR0, invCR1
